# revision 9
# baseline (speedup 1.0000x reference)
"""CSR Linear kernel for TRN2: out = x @ W^T + bias, W from COO nonzeros.

Strategy: data-parallel over tokens across 8 NeuronCores. Host densifies the
sparse weight into WT[in, out] (duplicate coords summed, bf16) and transposes
x (bf16); each core computes its 1024-token shard with a tiled bf16 matmul:
WT streamed from HBM once, x^T resident in SBUF, bias (host-replicated to all
128 partitions) fused into the PSUM->SBUF eviction. bf16 stationary operands
get the fast-weight-load path; half-size DMA keeps the early k-chunks ahead
of the PE so HAM reaches full clock quickly.
"""

import os
import sys
import types

import numpy as np

TOKENS = 8192
IN_F = 4096
OUT_F = 4096
N_CORES = 8
P = 128

_CACHE = {}


def _ensure_ntff_hook():
    """Register the axon NTFF profile hook if the antenv stub lacks it.

    Only needed when tracing (BASS_TRACE=1); harmless otherwise. In
    environments with a real antenv.axon_hooks this is a no-op.
    """
    try:
        import antenv.axon_hooks  # noqa: F401

        return
    except ImportError:
        pass
    try:
        import antenv
        from trn_agent_boot.trn_boot import _ntff_profile_via_ctypes

        hooks = types.ModuleType("antenv.axon_hooks")
        hooks._hook = _ntff_profile_via_ctypes("/opt/axon/libaxon_pjrt.so")
        hooks.set_axon_ntff_profile_hook = lambda h: setattr(hooks, "_hook", h)
        hooks.get_axon_ntff_profile_hook = lambda: hooks._hook
        sys.modules["antenv.axon_hooks"] = hooks
        antenv.axon_hooks = hooks
    except Exception:
        pass


def _patch_upload():
    """Make trace artifact upload fall back to the local tmpdir when no
    artifact bucket is reachable (container environments)."""
    from concourse import bass_utils

    orig = bass_utils.upload_artifacts
    if getattr(orig, "_kernel_patched", False):
        return

    def _safe_upload(tmpdir):
        try:
            return orig(tmpdir)
        except Exception:
            return tmpdir

    _safe_upload._kernel_patched = True
    bass_utils.upload_artifacts = _safe_upload


def build_program(tok_per_core=TOKENS // N_CORES, in_f=IN_F, out_f=OUT_F):
    """Build + compile the per-core Bass program.

    out[tok_per_core, out_f] = xt.T @ wt + bias, with
      xt [in_f, tok_per_core] (bf16), wt [in_f, out_f] (bf16),
      biasr [128, out_f] (f32, pre-replicated across partitions on host).
    """
    key = (tok_per_core, in_f, out_f)
    if key in _CACHE:
        return _CACHE[key]

    import concourse.bacc as bacc
    import concourse.mybir as mybir
    import concourse.tile as tile

    N_TILE = 512  # out-feature block per psum bank
    KO = in_f // P  # k tiles
    M = tok_per_core // P  # token tiles
    NB = out_f // N_TILE  # out-feature blocks
    KO_CHUNK0 = 8  # k-tiles per WT DMA in block 0 (fine-grained for the ramp)
    KO_CHUNK = 16  # k-tiles per WT DMA in blocks >= 1 (fewer boundary bubbles)
    WARMUP_MMS = 14  # keep the PE busy (HAM at 2.4 GHz) across the ~9us DMA spin-up

    nc = bacc.Bacc("TRN2", target_bir_lowering=False, debug=False)

    xt = nc.dram_tensor("xt", [in_f, tok_per_core], mybir.dt.bfloat16, kind="ExternalInput")
    wt = nc.dram_tensor("wt", [in_f, out_f], mybir.dt.bfloat16, kind="ExternalInput")
    biasr = nc.dram_tensor("biasr", [P, out_f], mybir.dt.float32, kind="ExternalInput")
    out = nc.dram_tensor("out", [tok_per_core, out_f], mybir.dt.float32, kind="ExternalOutput")

    xt_ap = xt.ap().rearrange("(ko p) t -> p ko t", p=P)  # [P, KO, T]
    wt_ap = wt.ap().rearrange("(ko p) o -> p ko o", p=P)  # [P, KO, out_f]
    out_ap = out.ap().rearrange("(mo p) o -> p mo o", p=P)  # [P, M, out_f]

    with tile.TileContext(nc) as tc:
        with (
            tc.tile_pool(name="xt_pool", bufs=1) as xt_pool,
            tc.tile_pool(name="bias_pool", bufs=1) as bias_pool,
            tc.tile_pool(name="warm_pool", bufs=1) as warm_pool,
            tc.tile_pool(name="wt0_pool", bufs=3) as wt0_pool,
            tc.tile_pool(name="wt_pool", bufs=4) as wt_pool,
            tc.tile_pool(name="out_pool", bufs=4) as out_pool,
            tc.tile_pool(name="psum", bufs=8, space="PSUM") as psum_pool,
        ):
            xt_sb = xt_pool.tile([P, KO, tok_per_core], mybir.dt.bfloat16)
            bias_sb = bias_pool.tile([P, out_f], mybir.dt.float32)

            # Warmup: the DMA rings deliver nothing for the first ~9us of the
            # NEFF, and the HAM clock gate drops the PE to 1.2 GHz after any
            # >3.4us idle window. Run throwaway matmuls on a zeroed tile so
            # the PE is warm (2.4 GHz) the moment real data lands; the first
            # real matmul of each accumulation group clears its PSUM bank via
            # start=True, so the garbage never escapes.
            wz = warm_pool.tile([P, N_TILE], mybir.dt.bfloat16)
            nc.gpsimd.memset(wz[:], 0.0)
            wps = psum_pool.tile([P, N_TILE], mybir.dt.float32, name="warm_ps", tag="ps")
            for i in range(WARMUP_MMS):
                # One accumulation chain: independent start=True matmuls into
                # the same bank serialize on the drain (~1.1us each).
                nc.tensor.matmul(
                    wps[:],
                    lhsT=wz[:, :P],
                    rhs=wz[:],
                    start=(i == 0),
                    stop=(i == WARMUP_MMS - 1),
                )

            def bounds(first, step):
                b = [0, min(first, KO)]
                while b[-1] + step < KO:
                    b.append(b[-1] + step)
                if b[-1] < KO:
                    b.append(KO)
                return list(zip(b[:-1], b[1:]))

            wt_chunks = {
                n: bounds(2 if n == 0 else KO_CHUNK, KO_CHUNK0 if n == 0 else KO_CHUNK)
                for n in range(NB)
            }
            xt_chunks = bounds(1, 4)  # 1 MiB bf16 chunks after a small lead-in

            def load_wt(n, kb, kbe):
                ns = slice(n * N_TILE, (n + 1) * N_TILE)
                pool, cap, tag = (
                    (wt0_pool, KO_CHUNK0, "wt0") if n == 0 else (wt_pool, KO_CHUNK, "wt")
                )
                wt_t = pool.tile(
                    [P, cap, N_TILE],
                    mybir.dt.bfloat16,
                    name=f"wt_{n}_{kb}",
                    tag=tag,
                )
                nc.sync.dma_start(wt_t[:, : kbe - kb, :], wt_ap[:, kb:kbe, ns])
                return wt_t

            def load_xt(j, je):
                return nc.sync.dma_start(xt_sb[:, j:je, :], xt_ap[:, j:je, :])

            # Emit block-0 WT chunks and x^T chunks interleaved in k-need
            # order so the DMA ramp delivers bytes in consumption order. The
            # bias load (2 MiB, first needed at block 0's eviction ~60us in)
            # is deliberately emitted after all of them.
            preloaded = {}
            xi = 0
            for kb, kbe in wt_chunks[0]:
                preloaded[(0, kb)] = load_wt(0, kb, kbe)
                while xi < len(xt_chunks) and xt_chunks[xi][0] < kbe + 4:
                    load_xt(*xt_chunks[xi])
                    xi += 1
            for j, je in xt_chunks[xi:]:
                load_xt(j, je)
            nc.sync.dma_start(bias_sb[:], biasr.ap())

            for n in range(NB):
                ns = slice(n * N_TILE, (n + 1) * N_TILE)
                # Prefetch the next block's WT chunks ahead of this block's
                # matmul stream: the Sync engine enqueues DMAs strictly in
                # program order, so chunks emitted lazily inside block n+1
                # would sit behind block n's eviction-gated out-DMAs and
                # arrive just-in-time (PE bubble per chunk).
                if n + 1 < NB:
                    for kb, kbe in wt_chunks[n + 1]:
                        preloaded[(n + 1, kb)] = load_wt(n + 1, kb, kbe)
                ps = [
                    psum_pool.tile(
                        [P, N_TILE], mybir.dt.float32, name=f"ps_{n}_{m}", tag="ps"
                    )
                    for m in range(M)
                ]
                for kb, kbe in wt_chunks[n]:
                    wt_t = preloaded.pop((n, kb), None)
                    if wt_t is None:
                        wt_t = load_wt(n, kb, kbe)
                    # k innermost: consecutive matmuls accumulate into the
                    # same PSUM bank (run length = chunk size) instead of
                    # cycling banks every matmul, which costs PE micro-idles.
                    for m in range(M):
                        for kk in range(kbe - kb):
                            ko = kb + kk
                            nc.tensor.matmul(
                                ps[m][:],
                                lhsT=xt_sb[:, ko, m * P : (m + 1) * P],
                                rhs=wt_t[:, kk, :],
                                start=(ko == 0),
                                stop=(ko == KO - 1),
                            )
                for m in range(M):
                    ot = out_pool.tile(
                        [P, N_TILE], mybir.dt.float32, name=f"ot_{n}_{m}", tag="ot"
                    )
                    nc.vector.tensor_add(out=ot[:], in0=ps[m][:], in1=bias_sb[:, ns])
                    nc.sync.dma_start(out_ap[:, m, ns], ot[:])

    nc.compile()
    _CACHE[key] = nc
    return nc


def _densify_wt(values, row_ids, col_ids, in_f=IN_F, out_f=OUT_F):
    """WT[i, o] = sum of values[k] over k with col_ids[k]==i, row_ids[k]==o."""
    idx = col_ids.astype(np.int64) * out_f + row_ids.astype(np.int64)
    wt = np.bincount(idx, weights=values.astype(np.float64), minlength=in_f * out_f)
    return np.ascontiguousarray(wt.astype(np.float32).reshape(in_f, out_f))


def kernel(x, values, row_ids, col_ids, bias):
    import ml_dtypes

    from concourse import bass_utils

    if os.environ.get("BASS_TRACE"):
        _ensure_ntff_hook()
        _patch_upload()

    nc = build_program()

    bf16 = ml_dtypes.bfloat16
    x = np.asarray(x, dtype=np.float32)
    values = np.asarray(values, dtype=np.float32)
    row_ids = np.asarray(row_ids)
    col_ids = np.asarray(col_ids)
    bias = np.asarray(bias, dtype=np.float32)

    wt = _densify_wt(values, row_ids, col_ids).astype(bf16)
    bias_rep = np.ascontiguousarray(
        np.broadcast_to(bias.astype(np.float32)[None, :], (P, OUT_F))
    )
    tpc = TOKENS // N_CORES
    in_maps = []
    for c in range(N_CORES):
        xt_c = np.ascontiguousarray(x[c * tpc : (c + 1) * tpc, :].T).astype(bf16)
        in_maps.append({"xt": xt_c, "wt": wt, "biasr": bias_rep})

    res = bass_utils.run_bass_kernel_spmd(nc, in_maps, core_ids=list(range(N_CORES)))
    global last_results
    last_results = res
    return np.concatenate([res.results[c]["out"] for c in range(N_CORES)], axis=0)


last_results = None
